# revision 12
# baseline (speedup 1.0000x reference)
"""Trainium2 Bass kernel for GQA attention (B=2, S=1024, HID=4096, H=32,
HKV=8, HD=128) with NeoX rotary + additive mask, sharded over 8 NeuronCores.

Sharding: 8-way tensor parallel. Each core owns 4 q heads + 1 kv head and
processes ALL 2048 tokens (both batches). wq/wk/wv column-sharded, wo
COLUMN-sharded (each core computes a disjoint 512-wide column stripe of the
output for all tokens). The only collective is a small bf16 AllGather of the
attention output (512KB per rank per 512-token block), fired per block and
overlapped with the remaining attention + wo compute. The host concatenates
the 8 disjoint column stripes.

Everything on device runs in a transposed layout ([feature, token]) so every
matmul streams with a wide free dim at full PE rate (fp32r for attention,
bf16 inputs for the projections with fp32 PSUM accumulation).
"""

import math

import ml_dtypes
import numpy as np

B, S, HID, H, HKV, HD = 2, 1024, 4096, 32, 8, 128
ST = B * S                   # total tokens
NCORES = 8
HL = H // NCORES             # q heads per core (4)
KVL = HKV // NCORES          # kv heads per core (1)
GQ = H // HKV                # q heads per kv head (4)
SCALE = 1.0 / math.sqrt(HD)
QB = 512                     # q block (free dim of attention matmuls)
XB = 256                     # token chunk for the QKV projection phase
NOUT = HID // NCORES         # output column stripe per core (512)
NTB = ST // QB               # 512-token blocks over all tokens (4)
KT = HID // 128              # contraction tiles (32)
NEG_THRESH = -1.0e8          # mask values <= this count as fully masked

_STATE: dict = {}


# ----------------------------------------------------------------------------
# walrus compat: this toolchain supports at most ONE semaphore wait per
# instruction; Tile's scheduler can attach several. Hoist extras onto
# same-engine nops placed immediately before the instruction.
# ----------------------------------------------------------------------------
def _split_multi_waits(nc):
    import concourse.mybir as mybir

    def detached_nop(engine_type):
        bi = nc.engines[engine_type].nop()
        inst = bi.ins
        for fn in nc.m.functions:
            for b in fn.blocks:
                il = b.instructions
                if il and il[-1].name == inst.name:
                    il.pop()
                    return inst
        raise AssertionError("could not detach nop")

    for fn in nc.m.functions:
        for b in fn.blocks:
            il = b.instructions
            out = []
            changed = False
            for inst in il:
                si = inst.sync_info
                waits = list(si.on_wait) if (si is not None and si.on_wait) else []
                if len(waits) > 1:
                    for w in waits[:-1]:
                        nop = detached_nop(inst.engine)
                        nop.sync_info = mybir.SyncInfo(on_wait=[w], on_update=[])
                        out.append(nop)
                    si.on_wait = waits[-1:]
                    changed = True
                out.append(inst)
            if changed:
                b.instructions = out


# ----------------------------------------------------------------------------
# Device program
# ----------------------------------------------------------------------------
def _build_module(mask_desc):
    """mask_desc: per (qb, kb) block descriptor list computed on the host from
    the actual attn_mask (same for both batches):
      ("skip",)                 block fully masked
      ("full", need_mask:bool)  full 512-wide block, optionally + mask data
      ("causal", off:int)       causal window: cols [off,512) active, mask
                                add on the 128-wide diagonal window at `off`
    """
    import concourse.bass as bass
    import concourse.mybir as mybir
    import concourse.tile as tile
    from concourse.masks import make_identity

    dt = mybir.dt
    f32, f32r, bf16 = dt.float32, dt.float32r, dt.bfloat16

    nc = bass.Bass()

    # mask blocks actually referenced by the program, in transposed [kv, q]
    # layout; index map built below.
    mask_tiles = []
    for qb in range(S // QB):
        for kb in range(S // 128):
            d = mask_desc[qb][kb]
            if d[0] == "causal":
                mask_tiles.append((qb, kb, 128))
            elif d[0] == "full" and d[1]:
                mask_tiles.append((qb, kb, QB))
    nmask = max(1, len(mask_tiles))
    mw = max([t[2] for t in mask_tiles], default=128)

    # --- DRAM parameters: everything is packed into two flat blobs (one per
    # dtype) so each repeat execution dispatches only 3 jit args instead of 9
    # (host-side dispatch cost scales with arg x shard count). Segment order
    # must match _prep_core_inputs.
    n_xt = KT * 128 * ST
    n_wq = HL * 128 * KT * 128
    n_wkv = KVL * 128 * KT * 128
    n_wo = KT * 128 * NOUT
    o_wq = n_xt
    o_wk = o_wq + n_wq
    o_wv = o_wk + n_wkv
    o_wo = o_wv + n_wkv
    NB16 = o_wo + n_wo
    n_cs = 128 * ST
    n_mask = nmask * 128 * mw
    o_sin = n_cs
    o_mask = 2 * n_cs
    NB32 = o_mask + n_mask

    b16 = nc.declare_dram_parameter("b16", [NB16], bf16, isOutput=False)
    b32 = nc.declare_dram_parameter("b32", [NB32], f32, isOutput=False)
    out_ext = nc.declare_dram_parameter("outp", [ST, NOUT], f32, isOutput=True)

    xt_in = b16[0:n_xt].rearrange("(k p t) -> k p t", k=KT, p=128, t=ST)
    wq_in = b16[o_wq:o_wq + n_wq].rearrange(
        "(h p k c) -> h p k c", h=HL, p=128, k=KT, c=128)
    wk_in = b16[o_wk:o_wk + n_wkv].rearrange(
        "(h p k c) -> h p k c", h=KVL, p=128, k=KT, c=128)
    wv_in = b16[o_wv:o_wv + n_wkv].rearrange(
        "(h p k c) -> h p k c", h=KVL, p=128, k=KT, c=128)
    wo_in = b16[o_wo:o_wo + n_wo].rearrange(
        "(k p n) -> k p n", k=KT, p=128, n=NOUT)
    cos_in = b32[0:n_cs].rearrange("(p t) -> p t", p=128, t=ST)
    sin_in = b32[o_sin:o_sin + n_cs].rearrange("(p t) -> p t", p=128, t=ST)
    mask_in = b32[o_mask:o_mask + n_mask].rearrange(
        "(b p c) -> b p c", b=nmask, p=128, c=mw)

    from contextlib import ExitStack
    ctx = ExitStack()
    with tile.TileContext(nc) as tc:
        const = ctx.enter_context(tc.tile_pool(name="const", bufs=1))
        persist = ctx.enter_context(tc.tile_pool(name="persist", bufs=1))
        dram = ctx.enter_context(tc.tile_pool(name="dram", bufs=1, space="DRAM"))

        ag_in = [dram.tile([HL * 128, QB], bf16, tag=f"agi{t}", name=f"ag_in{t}")
                 for t in range(NTB)]
        ag_out = [dram.tile([HID, QB], bf16, tag=f"ago{t}", name=f"ag_out{t}",
                            addr_space="Shared")
                  for t in range(NTB)]

        mask_idx = {(qb, kb): i for i, (qb, kb, _) in enumerate(mask_tiles)}

        # activations that live through phase 2 (freed before phase 3)
        q_rot = [persist.tile([128, ST], bf16, tag=f"q{h}", name=f"q_rot{h}")
                 for h in range(HL)]
        k_rot = persist.tile([128, ST], bf16, tag="k0", name="k_rot0")
        v_nat = persist.tile([128, ST // 128, 128], bf16, tag="v0", name="v_nat0")
        attn = persist.tile([128, HL, ST], bf16, tag="attn")

        # ---------------- phase 1: QKV projections + rotary -----------------
        with tc.tile_pool(name="p1x", bufs=2) as xpool, \
             tc.tile_pool(name="p1w", bufs=1) as wpool, \
             tc.tile_pool(name="p1t", bufs=2) as tpool, \
             tc.tile_pool(name="p1ps", bufs=2, space="PSUM") as pspool, \
             tc.tile_pool(name="p1pst", bufs=2, space="PSUM") as pstr:

            # all QKV weight tiles resident (6 x 1MB); issued first so the
            # tensor engine's first matmul is gated on as little DMA as
            # possible.
            w_sb = []  # (tile, kind, idx)
            for w_dram, ncts, kind in ((wk_in, KVL, "k"), (wv_in, KVL, "v"),
                                       (wq_in, HL, "q")):
                for ct in range(ncts):
                    w = wpool.tile([128, KT, 128], bf16, tag=f"w_{kind}{ct}",
                                   name=f"w_{kind}{ct}")
                    nc.sync.dma_start(out=w[:], in_=w_dram[ct])
                    w_sb.append((w, kind, ct))

            def load_chunk(xc):
                t = xpool.tile([128, KT, XB], bf16, tag="xt", name="xt")
                nc.sync.dma_start(
                    out=t[:],
                    in_=xt_in[:, :, xc * XB:(xc + 1) * XB]
                        .rearrange("k p t -> p k t"),
                )
                return t

            xt0 = load_chunk(0)

            cos_t = const.tile([128, ST], f32, tag="cos")
            sin_t = const.tile([128, ST], f32, tag="sin")
            nc.sync.dma_start(out=cos_t[:], in_=cos_in[:])
            nc.sync.dma_start(out=sin_t[:], in_=sin_in[:])
            ones32 = const.tile([128, 128], f32, tag="ones32")
            nc.gpsimd.memset(ones32[:], 1.0)
            ones_t = const.tile([128, 128], f32r, tag="ones")
            nc.vector.tensor_copy(ones_t[:], ones32[:])
            ident = const.tile([128, 128], f32, tag="ident")
            make_identity(nc, ident[:])
            mask_sb = const.tile([128, nmask, mw], f32, tag="mask")
            nc.sync.dma_start(out=mask_sb[:],
                              in_=mask_in[:].rearrange("b p c -> p b c"))
            for xc in range(ST // XB):
                xt = xt0 if xc == 0 else load_chunk(xc)
                tsl = slice(xc * XB, (xc + 1) * XB)
                for w, kind, ct in w_sb:
                    ps = pspool.tile([128, XB], f32, tag="ps_qkv")
                    for kt in range(KT):
                        nc.tensor.matmul(
                            ps[:],
                            w[:, kt, :],
                            xt[:, kt, :],
                            start=(kt == 0),
                            stop=(kt == KT - 1),
                        )
                    if kind in ("q", "k"):
                        dest = q_rot[ct] if kind == "q" else k_rot
                        swap = tpool.tile([128, XB], f32, tag="swap")
                        nc.scalar.activation(
                            swap[0:64, :], ps[64:128, :],
                            mybir.ActivationFunctionType.Copy, scale=-1.0)
                        nc.scalar.activation(
                            swap[64:128, :], ps[0:64, :],
                            mybir.ActivationFunctionType.Copy)
                        t2 = tpool.tile([128, XB], f32, tag="t2")
                        nc.vector.tensor_tensor(
                            t2[:], ps[:], cos_t[:, tsl], mybir.AluOpType.mult)
                        t3 = tpool.tile([128, XB], f32, tag="t3")
                        nc.vector.tensor_tensor(
                            t3[:], swap[:], sin_t[:, tsl], mybir.AluOpType.mult)
                        nc.vector.tensor_tensor(
                            dest[:, tsl], t2[:], t3[:], mybir.AluOpType.add)
                    else:  # v: transpose to natural [t, d] layout
                        vt = tpool.tile([128, XB], f32, tag="vt")
                        nc.scalar.activation(
                            vt[:], ps[:], mybir.ActivationFunctionType.Copy)
                        for j in range(XB // 128):
                            ps_t = pstr.tile([128, 128], f32, tag="ps_tr")
                            nc.tensor.transpose(
                                ps_t[:], vt[:, j * 128:(j + 1) * 128], ident[:])
                            nc.vector.tensor_copy(
                                v_nat[:, xc * (XB // 128) + j, :], ps_t[:])

        # ------- phases 2+3: attention + AllGather + output projection,
        # interleaved per 512-token block so each AG overlaps later compute.
        with tc.tile_pool(name="p2p", bufs=3) as ppool, \
             tc.tile_pool(name="p2r", bufs=2) as rpool, \
             tc.tile_pool(name="p2pa", bufs=2) as papool, \
             tc.tile_pool(name="p23w", bufs=1) as wopool, \
             tc.tile_pool(name="p3a", bufs=2) as agpool, \
             tc.tile_pool(name="p3o", bufs=4) as opool, \
             tc.tile_pool(name="p2sc", bufs=2, space="PSUM") as scpool, \
             tc.tile_pool(name="p2pv", bufs=2, space="PSUM") as pvpool, \
             tc.tile_pool(name="p2dn", bufs=2, space="PSUM") as dnpool, \
             tc.tile_pool(name="p3ps", bufs=2, space="PSUM") as pso:

            # resident bf16 wo column stripe [p(row within kt), kt, col]
            wo_sb = wopool.tile([128, KT, NOUT], bf16, tag="wo")
            nc.sync.dma_start(
                out=wo_sb[:], in_=wo_in[:, :, :].rearrange("k p n -> p k n"))

            def attention_block(tb):
                b, qb = divmod(tb, S // QB)
                for h in range(HL):
                    blocks = []  # (kb, off, mask_kind)
                    for kb in range(S // 128):
                        d = mask_desc[qb][kb]
                        if d[0] == "skip":
                            continue
                        if d[0] == "causal":
                            blocks.append((kb, d[1], ("diag", d[1])))
                        else:
                            blocks.append((kb, 0, ("full",) if d[1] else None))
                    ps_pv = pvpool.tile([128, QB], f32, tag="ps_pv")
                    nblk = len(blocks)
                    assert blocks[0][1] == 0, "first active block must be full-width"
                    p_acc = papool.tile([128, QB], f32r, tag="p_acc")
                    for bi, (kb, off, mk) in enumerate(blocks):
                        qsl = slice(b * S + qb * QB + off, b * S + (qb + 1) * QB)
                        kvo = b * (S // 128) + kb
                        ps_sc = scpool.tile([128, QB], f32, tag="ps_sc")
                        nc.tensor.matmul(
                            ps_sc[:, off:QB],
                            k_rot[:, kvo * 128:(kvo + 1) * 128],
                            q_rot[h][:, qsl],
                            start=True, stop=True,
                        )
                        if mk is not None:
                            mi = mask_idx[(qb, kb)]
                            if mk[0] == "diag":
                                nc.vector.tensor_tensor(
                                    ps_sc[:, off:off + 128], ps_sc[:, off:off + 128],
                                    mask_sb[:, mi, 0:128], mybir.AluOpType.add)
                            else:
                                nc.vector.tensor_tensor(
                                    ps_sc[:, 0:QB], ps_sc[:, 0:QB],
                                    mask_sb[:, mi, 0:QB], mybir.AluOpType.add)
                        p_t = ppool.tile([128, QB], bf16, tag="p")
                        nc.scalar.activation(
                            p_t[:, off:QB], ps_sc[:, off:QB],
                            mybir.ActivationFunctionType.Exp)
                        nc.tensor.matmul(
                            ps_pv[:, off:QB],
                            v_nat[:, kvo, :],
                            p_t[:, off:QB],
                            start=(bi == 0), stop=(bi == nblk - 1),
                        )
                        # denominator: accumulate p on the (otherwise idle)
                        # gpsimd engine; one ones-matmul per (unit, qb)
                        # afterwards instead of a full-width matmul per block.
                        if bi == 0:
                            nc.gpsimd.tensor_copy(p_acc[:], p_t[:])
                        else:
                            nc.gpsimd.tensor_tensor(
                                p_acc[:, off:QB], p_acc[:, off:QB],
                                p_t[:, off:QB], mybir.AluOpType.add)
                    ps_dn = dnpool.tile([128, QB], f32, tag="ps_dn")
                    nc.tensor.matmul(ps_dn[:], ones_t[:], p_acc[:],
                                     start=True, stop=True)
                    recip = rpool.tile([128, QB], f32, tag="recip")
                    nc.vector.reciprocal(recip[:], ps_dn[:])
                    nc.vector.tensor_tensor(
                        attn[:, h, tb * QB:(tb + 1) * QB], ps_pv[:], recip[:],
                        mybir.AluOpType.mult)
                # ship this block's attention columns for all local heads
                nc.sync.dma_start(
                    out=ag_in[tb].rearrange("(h p) t -> p h t", h=HL),
                    in_=attn[:, :, tb * QB:(tb + 1) * QB])
                nc.gpsimd.collective_compute(
                    "AllGather", mybir.AluOpType.bypass,
                    replica_groups=[list(range(NCORES))],
                    ins=[ag_in[tb].opt()], outs=[ag_out[tb].opt()],
                )

            def wo_block(tb):
                ag_sb = agpool.tile([128, KT, QB], bf16, tag="ag")
                nc.sync.dma_start(
                    out=ag_sb[:],
                    in_=ag_out[tb].rearrange("(k p) t -> p k t", k=KT))
                for tt in range(QB // 128):
                    ps_o = pso.tile([128, NOUT], f32, tag="ps_o")
                    for ct in range(KT):
                        nc.tensor.matmul(
                            ps_o[:],
                            ag_sb[:, ct, tt * 128:(tt + 1) * 128],
                            wo_sb[:, ct, :],
                            start=(ct == 0), stop=(ct == KT - 1),
                        )
                    o_sb = opool.tile([128, NOUT], f32, tag="o")
                    if tt % 2 == 0:
                        nc.vector.tensor_copy(o_sb[:], ps_o[:])
                    else:
                        nc.scalar.activation(
                            o_sb[:], ps_o[:], mybir.ActivationFunctionType.Copy)
                    nc.sync.dma_start(
                        out=out_ext[tb * QB + tt * 128:tb * QB + (tt + 1) * 128, :],
                        in_=o_sb[:])

            # schedule: wo(tb) is emitted two attention blocks after AG(tb)
            # fires so the gather latency hides under tensor work.
            for tb in range(NTB):
                attention_block(tb)
                if tb >= 2:
                    wo_block(tb - 2)
            wo_block(NTB - 2)
            wo_block(NTB - 1)

        ctx.close()

    _split_multi_waits(nc)
    return nc, [t[:2] for t in mask_tiles], mw


# ----------------------------------------------------------------------------
# Host-side input prep
# ----------------------------------------------------------------------------
def _classify_mask(attn_mask):
    """Per (qb, kb) descriptor from the actual mask contents (transposed
    [kv, q] view). Causal masks produce the efficient windowed structure."""
    mt = attn_mask.T  # [kv, q]
    desc = []
    for qb in range(S // QB):
        row = []
        q0 = qb * QB
        for kb in range(S // 128):
            blk = mt[kb * 128:(kb + 1) * 128, q0:q0 + QB]
            if np.all(blk <= NEG_THRESH):
                row.append(("skip",))
                continue
            if np.all(np.abs(blk) < 1e-6):
                row.append(("full", False))
                continue
            # causal window? cols [0, off) fully masked, diag at [off, off+128),
            # cols beyond fully visible
            off = kb * 128 - q0
            causal = False
            if 0 <= off <= QB - 128:
                left_ok = np.all(blk[:, :off] <= NEG_THRESH) if off else True
                right_ok = (np.all(np.abs(blk[:, off + 128:]) < 1e-6)
                            if off + 128 < QB else True)
                causal = bool(left_ok and right_ok)
            if causal:
                row.append(("causal", off))
            else:
                row.append(("full", True))
        desc.append(row)
    # every q column must keep at least one contributing block
    for qb in range(S // QB):
        assert any(d[0] != "skip" for d in desc[qb]), "fully-masked q rows unsupported"
    return desc


def _prep_core_inputs(inputs, mask_desc, mask_list, mw):
    x = np.asarray(inputs["x"], np.float32)
    wq = np.asarray(inputs["wq"], np.float32)
    wk = np.asarray(inputs["wk"], np.float32)
    wv = np.asarray(inputs["wv"], np.float32)
    wo = np.asarray(inputs["wo"], np.float32)
    attn_mask = np.asarray(inputs["attn_mask"], np.float32)
    start_pos = np.asarray(inputs["start_pos"], np.int32)

    bf = ml_dtypes.bfloat16

    inv_freq = 1.0 / (10000.0 ** (np.arange(0, HD, 2, dtype=np.float32) / HD))
    mt = attn_mask.T
    if mask_list:
        mask_arr = np.zeros((len(mask_list), 128, mw), np.float32)
        for i, (qb, kb) in enumerate(mask_list):
            d = mask_desc[qb][kb]
            if d[0] == "causal":
                off = d[1]
                mask_arr[i, :, 0:128] = mt[kb * 128:(kb + 1) * 128,
                                           qb * QB + off:qb * QB + off + 128]
            else:
                mask_arr[i, :, 0:QB] = mt[kb * 128:(kb + 1) * 128,
                                          qb * QB:(qb + 1) * QB]
    else:
        mask_arr = np.zeros((1, 128, mw), np.float32)

    # x transposed, all tokens; same for every core
    xt = np.ascontiguousarray(x.T.reshape(KT, 128, ST)).astype(bf)

    # rotary tables for all tokens (per-batch positions)
    pos = np.concatenate([start_pos[b] + np.arange(S, dtype=np.float32)
                          for b in range(B)])
    ang = pos[:, None] * inv_freq[None, :]              # [ST, HD/2]
    cos = np.concatenate([np.cos(ang), np.cos(ang)], -1).T  # [HD, ST]
    sin = np.concatenate([np.sin(ang), np.sin(ang)], -1).T

    # lhsT tile layout: [ct, p=hid_within_kt, kt, col_within_ct]
    def wtile2(w):
        c = w.shape[1]
        return np.ascontiguousarray(
            w.reshape(KT, 128, c // 128, 128).transpose(2, 1, 0, 3))

    b32 = np.concatenate([cos.astype(np.float32).ravel(),
                          sin.astype(np.float32).ravel(),
                          mask_arr.ravel()])
    b32 = np.ascontiguousarray(b32, np.float32)

    in_maps = []
    for core in range(NCORES):
        wq_c = (wq[:, core * HL * HD:(core + 1) * HL * HD] * SCALE)
        wk_c = wk[:, core * KVL * HD:(core + 1) * KVL * HD]
        wv_c = wv[:, core * KVL * HD:(core + 1) * KVL * HD]
        wo_c = wo[:, core * NOUT:(core + 1) * NOUT]      # [HID, 512]

        b16 = np.concatenate([
            xt.ravel(),
            wtile2(wq_c).astype(bf).ravel(),
            wtile2(wk_c).astype(bf).ravel(),
            wtile2(wv_c).astype(bf).ravel(),
            np.ascontiguousarray(wo_c.reshape(KT, 128, NOUT)).astype(bf).ravel(),
        ])
        in_maps.append({"b16": b16, "b32": b32})
    return in_maps


def _make_runner(nc):
    """Cached jit over the bass module (adapted from
    concourse.bass2jax.run_bass_via_pjrt so repeat calls reuse one NEFF)."""
    import jax
    import jax.numpy as jnp
    from jax.sharding import Mesh, NamedSharding, PartitionSpec
    from jax.experimental.shard_map import shard_map

    import concourse.mybir as mybir
    from concourse import bass2jax

    bass2jax.install_neuronx_cc_hook()
    assert nc.dbg_addr is None
    partition_name = (nc.partition_id_tensor.name
                      if nc.partition_id_tensor else None)

    in_names, out_names, out_avals, out_shapes = [], [], [], []
    for alloc in nc.m.functions[0].allocations:
        if not isinstance(alloc, mybir.MemoryLocationSet):
            continue
        name = alloc.memorylocations[0].name
        if alloc.kind == "ExternalInput":
            if name != partition_name:
                in_names.append(name)
        elif alloc.kind == "ExternalOutput":
            assert alloc.tensor_shape is not None and alloc.dtype is not None
            shape = tuple(alloc.tensor_shape)
            npdt = mybir.dt.np(alloc.dtype)
            out_names.append(name)
            out_shapes.append((shape, npdt))
            out_avals.append(jax.core.ShapedArray(shape, npdt))

    n_params = len(in_names)
    n_outs = len(out_names)
    all_in_names = in_names + out_names
    if partition_name is not None:
        all_in_names = all_in_names + [partition_name]
    donate = tuple(range(n_params, n_params + n_outs))

    def _body(*args):
        operands = list(args)
        if partition_name is not None:
            operands.append(bass2jax.partition_id_tensor())
        outs = bass2jax._bass_exec_p.bind(
            *operands,
            out_avals=tuple(out_avals),
            in_names=tuple(all_in_names),
            out_names=tuple(out_names),
            lowering_input_output_aliases=(),
            sim_require_finite=True,
            sim_require_nnan=True,
            nc=nc,
        )
        return tuple(outs)

    devices = jax.devices()[:NCORES]
    mesh = Mesh(np.asarray(devices), ("core",))
    pc = PartitionSpec("core")
    sharded = jax.jit(
        shard_map(_body, mesh=mesh, in_specs=(pc,) * (n_params + n_outs),
                  out_specs=(pc,) * n_outs, check_rep=False),
        donate_argnums=donate, keep_unused=True)

    shard_dev = NamedSharding(mesh, pc)

    def make_zeros():
        return tuple(
            jax.device_put(np.zeros((NCORES * s[0], *s[1:]), d), shard_dev)
            for s, d in out_shapes)

    def put_inputs(in_maps):
        return [
            jax.device_put(
                np.concatenate([np.asarray(m[nm]) for m in in_maps], axis=0),
                shard_dev)
            for nm in in_names]

    def run_from_dev(in_dev, zeros):
        out_arrs = sharded(*in_dev, *zeros)
        jax.block_until_ready(out_arrs)
        return out_arrs

    def run(in_maps):
        out_arrs = run_from_dev(put_inputs(in_maps), make_zeros())
        return [
            {nm: np.asarray(out_arrs[i]).reshape(NCORES, *out_shapes[i][0])[c]
             for i, nm in enumerate(out_names)}
            for c in range(NCORES)]

    return {"run": run, "put_inputs": put_inputs, "make_zeros": make_zeros,
            "run_from_dev": run_from_dev, "sharded": sharded}


def _get_runner(mask_desc):
    key = repr(mask_desc)
    if _STATE.get("key") == key:
        return _STATE["run"], _STATE["mask_list"], _STATE["mw"]

    nc, mask_list, mw = _build_module(mask_desc)
    runner = _make_runner(nc)

    _STATE.update({"key": key, "run": runner["run"], "mask_list": mask_list,
                   "mw": mw, "nc": nc, "runner": runner})
    return runner["run"], mask_list, mw


def kernel(**inputs) -> np.ndarray:
    attn_mask = np.asarray(inputs["attn_mask"], np.float32)
    mask_desc = _classify_mask(attn_mask)
    run, mask_list, mw = _get_runner(mask_desc)
    in_maps = _prep_core_inputs(inputs, mask_desc, mask_list, mw)
    results = run(in_maps)
    out = np.empty((ST, HID), np.float32)
    for core in range(NCORES):
        out[:, core * NOUT:(core + 1) * NOUT] = results[core]["outp"]
    return out


# revision 13
# speedup vs baseline: 1.9152x; 1.9152x over previous
"""Trainium2 Bass kernel for GQA attention (B=2, S=1024, HID=4096, H=32,
HKV=8, HD=128) with NeoX rotary + additive mask, sharded over 8 NeuronCores.

Sharding: 8-way tensor parallel. Each core owns 4 q heads + 1 kv head and
processes ALL 2048 tokens (both batches). wq/wk/wv column-sharded, wo
COLUMN-sharded (each core computes a disjoint 512-wide column stripe of the
output for all tokens). The only collective is a small bf16 AllGather of the
attention output (512KB per rank per 512-token block), fired per block and
overlapped with the remaining attention + wo compute. The host concatenates
the 8 disjoint column stripes.

Everything on device runs in a transposed layout ([feature, token]) so every
matmul streams with a wide free dim at full PE rate (fp32r for attention,
bf16 inputs for the projections with fp32 PSUM accumulation).
"""

import math

import ml_dtypes
import numpy as np

B, S, HID, H, HKV, HD = 2, 1024, 4096, 32, 8, 128
ST = B * S                   # total tokens
NCORES = 8
HL = H // NCORES             # q heads per core (4)
KVL = HKV // NCORES          # kv heads per core (1)
GQ = H // HKV                # q heads per kv head (4)
SCALE = 1.0 / math.sqrt(HD)
QB = 512                     # q block (free dim of attention matmuls)
XB = 512                     # token chunk for the QKV projection phase
NOUT = HID // NCORES         # output column stripe per core (512)
NTB = ST // QB               # 512-token blocks over all tokens (4)
KT = HID // 128              # contraction tiles (32)
NEG_THRESH = -1.0e8          # mask values <= this count as fully masked

_STATE: dict = {}


# ----------------------------------------------------------------------------
# walrus compat: this toolchain supports at most ONE semaphore wait per
# instruction; Tile's scheduler can attach several. Hoist extras onto
# same-engine nops placed immediately before the instruction.
# ----------------------------------------------------------------------------
def _split_multi_waits(nc):
    import concourse.mybir as mybir

    def detached_nop(engine_type):
        bi = nc.engines[engine_type].nop()
        inst = bi.ins
        for fn in nc.m.functions:
            for b in fn.blocks:
                il = b.instructions
                if il and il[-1].name == inst.name:
                    il.pop()
                    return inst
        raise AssertionError("could not detach nop")

    for fn in nc.m.functions:
        for b in fn.blocks:
            il = b.instructions
            out = []
            changed = False
            for inst in il:
                si = inst.sync_info
                waits = list(si.on_wait) if (si is not None and si.on_wait) else []
                if len(waits) > 1:
                    for w in waits[:-1]:
                        nop = detached_nop(inst.engine)
                        nop.sync_info = mybir.SyncInfo(on_wait=[w], on_update=[])
                        out.append(nop)
                    si.on_wait = waits[-1:]
                    changed = True
                out.append(inst)
            if changed:
                b.instructions = out


# ----------------------------------------------------------------------------
# Device program
# ----------------------------------------------------------------------------
def _build_module(mask_desc):
    """mask_desc: per (qb, kb) block descriptor list computed on the host from
    the actual attn_mask (same for both batches):
      ("skip",)                 block fully masked
      ("full", need_mask:bool)  full 512-wide block, optionally + mask data
      ("causal", off:int)       causal window: cols [off,512) active, mask
                                add on the 128-wide diagonal window at `off`
    """
    import concourse.bass as bass
    import concourse.mybir as mybir
    import concourse.tile as tile
    from concourse.masks import make_identity

    dt = mybir.dt
    f32, f32r, bf16 = dt.float32, dt.float32r, dt.bfloat16

    nc = bass.Bass()

    # mask blocks actually referenced by the program, in transposed [kv, q]
    # layout; index map built below.
    mask_tiles = []
    for qb in range(S // QB):
        for kb in range(S // 128):
            d = mask_desc[qb][kb]
            if d[0] == "causal":
                mask_tiles.append((qb, kb, 128))
            elif d[0] == "full" and d[1]:
                mask_tiles.append((qb, kb, QB))
    nmask = max(1, len(mask_tiles))
    mw = max([t[2] for t in mask_tiles], default=128)

    # --- DRAM parameters: everything is packed into two flat blobs (one per
    # dtype) so each repeat execution dispatches only 3 jit args instead of 9
    # (host-side dispatch cost scales with arg x shard count). Segment order
    # must match _prep_core_inputs.
    n_xt = KT * 128 * ST
    n_wq = HL * 128 * KT * 128
    n_wkv = KVL * 128 * KT * 128
    n_wo = KT * 128 * NOUT
    o_wq = n_xt
    o_wk = o_wq + n_wq
    o_wv = o_wk + n_wkv
    o_wo = o_wv + n_wkv
    NB16 = o_wo + n_wo
    n_cs = 128 * ST
    n_mask = nmask * 128 * mw
    o_sin = n_cs
    o_mask = 2 * n_cs
    NB32 = o_mask + n_mask

    b16 = nc.declare_dram_parameter("b16", [NB16], bf16, isOutput=False)
    b32 = nc.declare_dram_parameter("b32", [NB32], f32, isOutput=False)
    out_ext = nc.declare_dram_parameter("outp", [ST, NOUT], f32, isOutput=True)

    xt_in = b16[0:n_xt].rearrange("(k p t) -> k p t", k=KT, p=128, t=ST)
    wq_in = b16[o_wq:o_wq + n_wq].rearrange(
        "(h p k c) -> h p k c", h=HL, p=128, k=KT, c=128)
    wk_in = b16[o_wk:o_wk + n_wkv].rearrange(
        "(h p k c) -> h p k c", h=KVL, p=128, k=KT, c=128)
    wv_in = b16[o_wv:o_wv + n_wkv].rearrange(
        "(h p k c) -> h p k c", h=KVL, p=128, k=KT, c=128)
    wo_in = b16[o_wo:o_wo + n_wo].rearrange(
        "(k p n) -> k p n", k=KT, p=128, n=NOUT)
    cos_in = b32[0:n_cs].rearrange("(p t) -> p t", p=128, t=ST)
    sin_in = b32[o_sin:o_sin + n_cs].rearrange("(p t) -> p t", p=128, t=ST)
    mask_in = b32[o_mask:o_mask + n_mask].rearrange(
        "(b p c) -> b p c", b=nmask, p=128, c=mw)

    from contextlib import ExitStack
    ctx = ExitStack()
    with tile.TileContext(nc) as tc:
        const = ctx.enter_context(tc.tile_pool(name="const", bufs=1))
        persist = ctx.enter_context(tc.tile_pool(name="persist", bufs=1))
        dram = ctx.enter_context(tc.tile_pool(name="dram", bufs=1, space="DRAM"))

        ag_in = [dram.tile([HL * 128, QB], bf16, tag=f"agi{t}", name=f"ag_in{t}")
                 for t in range(NTB)]
        ag_out = [dram.tile([HID, QB], bf16, tag=f"ago{t}", name=f"ag_out{t}",
                            addr_space="Shared")
                  for t in range(NTB)]

        mask_idx = {(qb, kb): i for i, (qb, kb, _) in enumerate(mask_tiles)}

        # activations that live through phase 2 (freed before phase 3)
        q_rot = [persist.tile([128, ST], bf16, tag=f"q{h}", name=f"q_rot{h}")
                 for h in range(HL)]
        k_rot = persist.tile([128, ST], bf16, tag="k0", name="k_rot0")
        v_nat = persist.tile([128, ST // 128, 128], bf16, tag="v0", name="v_nat0")
        attn = persist.tile([128, HL, ST], bf16, tag="attn")

        # ---------------- phase 1: QKV projections + rotary -----------------
        with tc.tile_pool(name="p1x", bufs=2) as xpool, \
             tc.tile_pool(name="p1w", bufs=1) as wpool, \
             tc.tile_pool(name="p1t", bufs=2) as tpool, \
             tc.tile_pool(name="p1ps", bufs=2, space="PSUM") as pspool, \
             tc.tile_pool(name="p1pst", bufs=2, space="PSUM") as pstr:

            # all QKV weight tiles resident (6 x 1MB); issued first so the
            # tensor engine's first matmul is gated on as little DMA as
            # possible.
            w_sb = []  # (tile, kind, idx)
            for w_dram, ncts, kind in ((wk_in, KVL, "k"), (wv_in, KVL, "v"),
                                       (wq_in, HL, "q")):
                for ct in range(ncts):
                    w = wpool.tile([128, KT, 128], bf16, tag=f"w_{kind}{ct}",
                                   name=f"w_{kind}{ct}")
                    nc.sync.dma_start(out=w[:], in_=w_dram[ct])
                    w_sb.append((w, kind, ct))

            def load_chunk(xc):
                t = xpool.tile([128, KT, XB], bf16, tag="xt", name="xt")
                nc.sync.dma_start(
                    out=t[:],
                    in_=xt_in[:, :, xc * XB:(xc + 1) * XB]
                        .rearrange("k p t -> p k t"),
                )
                return t

            xt0 = load_chunk(0)

            cos_t = const.tile([128, ST], f32, tag="cos")
            sin_t = const.tile([128, ST], f32, tag="sin")
            nc.sync.dma_start(out=cos_t[:], in_=cos_in[:])
            nc.sync.dma_start(out=sin_t[:], in_=sin_in[:])
            ones32 = const.tile([128, 128], f32, tag="ones32")
            nc.gpsimd.memset(ones32[:], 1.0)
            ones_t = const.tile([128, 128], f32r, tag="ones")
            nc.vector.tensor_copy(ones_t[:], ones32[:])
            ident = const.tile([128, 128], f32, tag="ident")
            make_identity(nc, ident[:])
            mask_sb = const.tile([128, nmask, mw], f32, tag="mask")
            nc.sync.dma_start(out=mask_sb[:],
                              in_=mask_in[:].rearrange("b p c -> p b c"))
            for xc in range(ST // XB):
                xt = xt0 if xc == 0 else load_chunk(xc)
                tsl = slice(xc * XB, (xc + 1) * XB)
                for w, kind, ct in w_sb:
                    ps = pspool.tile([128, XB], f32, tag="ps_qkv")
                    for kt in range(KT):
                        nc.tensor.matmul(
                            ps[:],
                            w[:, kt, :],
                            xt[:, kt, :],
                            start=(kt == 0),
                            stop=(kt == KT - 1),
                        )
                    if kind in ("q", "k"):
                        dest = q_rot[ct] if kind == "q" else k_rot
                        swap = tpool.tile([128, XB], f32, tag="swap")
                        nc.scalar.activation(
                            swap[0:64, :], ps[64:128, :],
                            mybir.ActivationFunctionType.Copy, scale=-1.0)
                        nc.scalar.activation(
                            swap[64:128, :], ps[0:64, :],
                            mybir.ActivationFunctionType.Copy)
                        t2 = tpool.tile([128, XB], f32, tag="t2")
                        nc.vector.tensor_tensor(
                            t2[:], ps[:], cos_t[:, tsl], mybir.AluOpType.mult)
                        t3 = tpool.tile([128, XB], f32, tag="t3")
                        nc.vector.tensor_tensor(
                            t3[:], swap[:], sin_t[:, tsl], mybir.AluOpType.mult)
                        nc.vector.tensor_tensor(
                            dest[:, tsl], t2[:], t3[:], mybir.AluOpType.add)
                    else:  # v: transpose to natural [t, d] layout
                        vt = tpool.tile([128, XB], f32, tag="vt")
                        nc.scalar.activation(
                            vt[:], ps[:], mybir.ActivationFunctionType.Copy)
                        for j in range(XB // 128):
                            ps_t = pstr.tile([128, 128], f32, tag="ps_tr")
                            nc.tensor.transpose(
                                ps_t[:], vt[:, j * 128:(j + 1) * 128], ident[:])
                            nc.vector.tensor_copy(
                                v_nat[:, xc * (XB // 128) + j, :], ps_t[:])

        # ------- phases 2+3: attention + AllGather + output projection,
        # interleaved per 512-token block so each AG overlaps later compute.
        with tc.tile_pool(name="p2p", bufs=3) as ppool, \
             tc.tile_pool(name="p2r", bufs=2) as rpool, \
             tc.tile_pool(name="p2pa", bufs=2) as papool, \
             tc.tile_pool(name="p23w", bufs=1) as wopool, \
             tc.tile_pool(name="p3a", bufs=2) as agpool, \
             tc.tile_pool(name="p3o", bufs=4) as opool, \
             tc.tile_pool(name="p2sc", bufs=2, space="PSUM") as scpool, \
             tc.tile_pool(name="p2pv", bufs=2, space="PSUM") as pvpool, \
             tc.tile_pool(name="p2dn", bufs=2, space="PSUM") as dnpool, \
             tc.tile_pool(name="p3ps", bufs=2, space="PSUM") as pso:

            # resident bf16 wo column stripe [p(row within kt), kt, col]
            wo_sb = wopool.tile([128, KT, NOUT], bf16, tag="wo")
            nc.sync.dma_start(
                out=wo_sb[:], in_=wo_in[:, :, :].rearrange("k p n -> p k n"))

            def attention_block(tb):
                b, qb = divmod(tb, S // QB)
                for h in range(HL):
                    blocks = []  # (kb, off, mask_kind)
                    for kb in range(S // 128):
                        d = mask_desc[qb][kb]
                        if d[0] == "skip":
                            continue
                        if d[0] == "causal":
                            blocks.append((kb, d[1], ("diag", d[1])))
                        else:
                            blocks.append((kb, 0, ("full",) if d[1] else None))
                    ps_pv = pvpool.tile([128, QB], f32, tag="ps_pv")
                    nblk = len(blocks)
                    assert blocks[0][1] == 0, "first active block must be full-width"
                    p_acc = papool.tile([128, QB], f32r, tag="p_acc")
                    for bi, (kb, off, mk) in enumerate(blocks):
                        qsl = slice(b * S + qb * QB + off, b * S + (qb + 1) * QB)
                        kvo = b * (S // 128) + kb
                        ps_sc = scpool.tile([128, QB], f32, tag="ps_sc")
                        nc.tensor.matmul(
                            ps_sc[:, off:QB],
                            k_rot[:, kvo * 128:(kvo + 1) * 128],
                            q_rot[h][:, qsl],
                            start=True, stop=True,
                        )
                        if mk is not None:
                            mi = mask_idx[(qb, kb)]
                            if mk[0] == "diag":
                                nc.vector.tensor_tensor(
                                    ps_sc[:, off:off + 128], ps_sc[:, off:off + 128],
                                    mask_sb[:, mi, 0:128], mybir.AluOpType.add)
                            else:
                                nc.vector.tensor_tensor(
                                    ps_sc[:, 0:QB], ps_sc[:, 0:QB],
                                    mask_sb[:, mi, 0:QB], mybir.AluOpType.add)
                        p_t = ppool.tile([128, QB], bf16, tag="p")
                        nc.scalar.activation(
                            p_t[:, off:QB], ps_sc[:, off:QB],
                            mybir.ActivationFunctionType.Exp)
                        nc.tensor.matmul(
                            ps_pv[:, off:QB],
                            v_nat[:, kvo, :],
                            p_t[:, off:QB],
                            start=(bi == 0), stop=(bi == nblk - 1),
                        )
                        # denominator: accumulate p on the (otherwise idle)
                        # gpsimd engine; one ones-matmul per (unit, qb)
                        # afterwards instead of a full-width matmul per block.
                        if bi == 0:
                            nc.gpsimd.tensor_copy(p_acc[:], p_t[:])
                        else:
                            nc.gpsimd.tensor_tensor(
                                p_acc[:, off:QB], p_acc[:, off:QB],
                                p_t[:, off:QB], mybir.AluOpType.add)
                    ps_dn = dnpool.tile([128, QB], f32, tag="ps_dn")
                    nc.tensor.matmul(ps_dn[:], ones_t[:], p_acc[:],
                                     start=True, stop=True)
                    recip = rpool.tile([128, QB], f32, tag="recip")
                    nc.vector.reciprocal(recip[:], ps_dn[:])
                    nc.vector.tensor_tensor(
                        attn[:, h, tb * QB:(tb + 1) * QB], ps_pv[:], recip[:],
                        mybir.AluOpType.mult)
                # ship this block's attention columns for all local heads
                nc.sync.dma_start(
                    out=ag_in[tb].rearrange("(h p) t -> p h t", h=HL),
                    in_=attn[:, :, tb * QB:(tb + 1) * QB])
                nc.gpsimd.collective_compute(
                    "AllGather", mybir.AluOpType.bypass,
                    replica_groups=[list(range(NCORES))],
                    ins=[ag_in[tb].opt()], outs=[ag_out[tb].opt()],
                )

            def wo_block(tb):
                ag_sb = agpool.tile([128, KT, QB], bf16, tag="ag")
                nc.sync.dma_start(
                    out=ag_sb[:],
                    in_=ag_out[tb].rearrange("(k p) t -> p k t", k=KT))
                for tt in range(QB // 128):
                    ps_o = pso.tile([128, NOUT], f32, tag="ps_o")
                    for ct in range(KT):
                        nc.tensor.matmul(
                            ps_o[:],
                            ag_sb[:, ct, tt * 128:(tt + 1) * 128],
                            wo_sb[:, ct, :],
                            start=(ct == 0), stop=(ct == KT - 1),
                        )
                    o_sb = opool.tile([128, NOUT], f32, tag="o")
                    if tt % 2 == 0:
                        nc.vector.tensor_copy(o_sb[:], ps_o[:])
                    else:
                        nc.scalar.activation(
                            o_sb[:], ps_o[:], mybir.ActivationFunctionType.Copy)
                    nc.sync.dma_start(
                        out=out_ext[tb * QB + tt * 128:tb * QB + (tt + 1) * 128, :],
                        in_=o_sb[:])

            # schedule: wo(tb) is emitted two attention blocks after AG(tb)
            # fires so the gather latency hides under tensor work.
            for tb in range(NTB):
                attention_block(tb)
                if tb >= 2:
                    wo_block(tb - 2)
            wo_block(NTB - 2)
            wo_block(NTB - 1)

        ctx.close()

    _split_multi_waits(nc)
    return nc, [t[:2] for t in mask_tiles], mw


# ----------------------------------------------------------------------------
# Host-side input prep
# ----------------------------------------------------------------------------
def _classify_mask(attn_mask):
    """Per (qb, kb) descriptor from the actual mask contents (transposed
    [kv, q] view). Causal masks produce the efficient windowed structure."""
    mt = attn_mask.T  # [kv, q]
    desc = []
    for qb in range(S // QB):
        row = []
        q0 = qb * QB
        for kb in range(S // 128):
            blk = mt[kb * 128:(kb + 1) * 128, q0:q0 + QB]
            if np.all(blk <= NEG_THRESH):
                row.append(("skip",))
                continue
            if np.all(np.abs(blk) < 1e-6):
                row.append(("full", False))
                continue
            # causal window? cols [0, off) fully masked, diag at [off, off+128),
            # cols beyond fully visible
            off = kb * 128 - q0
            causal = False
            if 0 <= off <= QB - 128:
                left_ok = np.all(blk[:, :off] <= NEG_THRESH) if off else True
                right_ok = (np.all(np.abs(blk[:, off + 128:]) < 1e-6)
                            if off + 128 < QB else True)
                causal = bool(left_ok and right_ok)
            if causal:
                row.append(("causal", off))
            else:
                row.append(("full", True))
        desc.append(row)
    # every q column must keep at least one contributing block
    for qb in range(S // QB):
        assert any(d[0] != "skip" for d in desc[qb]), "fully-masked q rows unsupported"
    return desc


def _prep_core_inputs(inputs, mask_desc, mask_list, mw):
    x = np.asarray(inputs["x"], np.float32)
    wq = np.asarray(inputs["wq"], np.float32)
    wk = np.asarray(inputs["wk"], np.float32)
    wv = np.asarray(inputs["wv"], np.float32)
    wo = np.asarray(inputs["wo"], np.float32)
    attn_mask = np.asarray(inputs["attn_mask"], np.float32)
    start_pos = np.asarray(inputs["start_pos"], np.int32)

    bf = ml_dtypes.bfloat16

    inv_freq = 1.0 / (10000.0 ** (np.arange(0, HD, 2, dtype=np.float32) / HD))
    mt = attn_mask.T
    if mask_list:
        mask_arr = np.zeros((len(mask_list), 128, mw), np.float32)
        for i, (qb, kb) in enumerate(mask_list):
            d = mask_desc[qb][kb]
            if d[0] == "causal":
                off = d[1]
                mask_arr[i, :, 0:128] = mt[kb * 128:(kb + 1) * 128,
                                           qb * QB + off:qb * QB + off + 128]
            else:
                mask_arr[i, :, 0:QB] = mt[kb * 128:(kb + 1) * 128,
                                          qb * QB:(qb + 1) * QB]
    else:
        mask_arr = np.zeros((1, 128, mw), np.float32)

    # x transposed, all tokens; same for every core
    xt = np.ascontiguousarray(x.T.reshape(KT, 128, ST)).astype(bf)

    # rotary tables for all tokens (per-batch positions)
    pos = np.concatenate([start_pos[b] + np.arange(S, dtype=np.float32)
                          for b in range(B)])
    ang = pos[:, None] * inv_freq[None, :]              # [ST, HD/2]
    cos = np.concatenate([np.cos(ang), np.cos(ang)], -1).T  # [HD, ST]
    sin = np.concatenate([np.sin(ang), np.sin(ang)], -1).T

    # lhsT tile layout: [ct, p=hid_within_kt, kt, col_within_ct]
    def wtile2(w):
        c = w.shape[1]
        return np.ascontiguousarray(
            w.reshape(KT, 128, c // 128, 128).transpose(2, 1, 0, 3))

    b32 = np.concatenate([cos.astype(np.float32).ravel(),
                          sin.astype(np.float32).ravel(),
                          mask_arr.ravel()])
    b32 = np.ascontiguousarray(b32, np.float32)

    in_maps = []
    for core in range(NCORES):
        wq_c = (wq[:, core * HL * HD:(core + 1) * HL * HD] * SCALE)
        wk_c = wk[:, core * KVL * HD:(core + 1) * KVL * HD]
        wv_c = wv[:, core * KVL * HD:(core + 1) * KVL * HD]
        wo_c = wo[:, core * NOUT:(core + 1) * NOUT]      # [HID, 512]

        b16 = np.concatenate([
            xt.ravel(),
            wtile2(wq_c).astype(bf).ravel(),
            wtile2(wk_c).astype(bf).ravel(),
            wtile2(wv_c).astype(bf).ravel(),
            np.ascontiguousarray(wo_c.reshape(KT, 128, NOUT)).astype(bf).ravel(),
        ])
        in_maps.append({"b16": b16, "b32": b32})
    return in_maps


def _make_runner(nc):
    """Cached jit over the bass module (adapted from
    concourse.bass2jax.run_bass_via_pjrt so repeat calls reuse one NEFF)."""
    import jax
    import jax.numpy as jnp
    from jax.sharding import Mesh, NamedSharding, PartitionSpec
    from jax.experimental.shard_map import shard_map

    import concourse.mybir as mybir
    from concourse import bass2jax

    bass2jax.install_neuronx_cc_hook()
    assert nc.dbg_addr is None
    partition_name = (nc.partition_id_tensor.name
                      if nc.partition_id_tensor else None)

    in_names, out_names, out_avals, out_shapes = [], [], [], []
    for alloc in nc.m.functions[0].allocations:
        if not isinstance(alloc, mybir.MemoryLocationSet):
            continue
        name = alloc.memorylocations[0].name
        if alloc.kind == "ExternalInput":
            if name != partition_name:
                in_names.append(name)
        elif alloc.kind == "ExternalOutput":
            assert alloc.tensor_shape is not None and alloc.dtype is not None
            shape = tuple(alloc.tensor_shape)
            npdt = mybir.dt.np(alloc.dtype)
            out_names.append(name)
            out_shapes.append((shape, npdt))
            out_avals.append(jax.core.ShapedArray(shape, npdt))

    n_params = len(in_names)
    n_outs = len(out_names)
    all_in_names = in_names + out_names
    if partition_name is not None:
        all_in_names = all_in_names + [partition_name]
    donate = tuple(range(n_params, n_params + n_outs))

    def _body(*args):
        operands = list(args)
        if partition_name is not None:
            operands.append(bass2jax.partition_id_tensor())
        outs = bass2jax._bass_exec_p.bind(
            *operands,
            out_avals=tuple(out_avals),
            in_names=tuple(all_in_names),
            out_names=tuple(out_names),
            lowering_input_output_aliases=(),
            sim_require_finite=True,
            sim_require_nnan=True,
            nc=nc,
        )
        return tuple(outs)

    devices = jax.devices()[:NCORES]
    mesh = Mesh(np.asarray(devices), ("core",))
    pc = PartitionSpec("core")
    sharded = jax.jit(
        shard_map(_body, mesh=mesh, in_specs=(pc,) * (n_params + n_outs),
                  out_specs=(pc,) * n_outs, check_rep=False),
        donate_argnums=donate, keep_unused=True)

    shard_dev = NamedSharding(mesh, pc)

    def make_zeros():
        return tuple(
            jax.device_put(np.zeros((NCORES * s[0], *s[1:]), d), shard_dev)
            for s, d in out_shapes)

    def put_inputs(in_maps):
        return [
            jax.device_put(
                np.concatenate([np.asarray(m[nm]) for m in in_maps], axis=0),
                shard_dev)
            for nm in in_names]

    def run_from_dev(in_dev, zeros):
        out_arrs = sharded(*in_dev, *zeros)
        jax.block_until_ready(out_arrs)
        return out_arrs

    def run(in_maps):
        out_arrs = run_from_dev(put_inputs(in_maps), make_zeros())
        return [
            {nm: np.asarray(out_arrs[i]).reshape(NCORES, *out_shapes[i][0])[c]
             for i, nm in enumerate(out_names)}
            for c in range(NCORES)]

    return {"run": run, "put_inputs": put_inputs, "make_zeros": make_zeros,
            "run_from_dev": run_from_dev, "sharded": sharded}


def _get_runner(mask_desc):
    key = repr(mask_desc)
    if _STATE.get("key") == key:
        return _STATE["run"], _STATE["mask_list"], _STATE["mw"]

    nc, mask_list, mw = _build_module(mask_desc)
    runner = _make_runner(nc)

    _STATE.update({"key": key, "run": runner["run"], "mask_list": mask_list,
                   "mw": mw, "nc": nc, "runner": runner})
    return runner["run"], mask_list, mw


def kernel(**inputs) -> np.ndarray:
    attn_mask = np.asarray(inputs["attn_mask"], np.float32)
    mask_desc = _classify_mask(attn_mask)
    run, mask_list, mw = _get_runner(mask_desc)
    in_maps = _prep_core_inputs(inputs, mask_desc, mask_list, mw)
    results = run(in_maps)
    out = np.empty((ST, HID), np.float32)
    for core in range(NCORES):
        out[:, core * NOUT:(core + 1) * NOUT] = results[core]["outp"]
    return out
